# revision 14
# baseline (speedup 1.0000x reference)
"""Trainium2 Bass kernel for nn_CriticGraphPolicy (gnn_message_passing).

Tree-structured critic: 20 limbs in a fixed binary tree, messages flow
root->leaves. Per limb: Q-net (65->128->128->1) and (non-leaf only)
message net (33->128->128->64 + l2norm).

Strategy: pure data-parallel over batch (32768 / 8 cores = 4096 each).
Activations are feature-major [feat, batch_tile] so every layer is a
natural PE matmul with the small shared weights as the stationary
operand.  Host-side prep packs state+action transposed per limb so no
on-chip transpose is needed.
"""

import numpy as np

import concourse.bass as bass
import concourse.tile as tile
from concourse import bacc, mybir
from concourse import bass_utils

F32 = mybir.dt.float32

NUM_LIMBS = 20
STATE_DIM = 32
MSG_DIM = 32
BATCH = 32768
N_CORES = 8
BC = BATCH // N_CORES          # batch per core
NT = 512                       # batch tile (one PSUM bank of fp32)

PARENTS = [-1] + [(i - 1) // 2 for i in range(1, NUM_LIMBS)]
CHILD_IDX = [0] + [(i - 1) % 2 for i in range(1, NUM_LIMBS)]
NON_LEAF = [i for i in range(NUM_LIMBS) if 2 * i + 1 < NUM_LIMBS]  # 0..9

# depth groups (topological levels of the tree)
def _depth(i):
    d = 0
    while PARENTS[i] >= 0:
        i = PARENTS[i]
        d += 1
    return d

DEPTHS = []
for i in range(NUM_LIMBS):
    d = _depth(i)
    while len(DEPTHS) <= d:
        DEPTHS.append([])
    DEPTHS[d].append(i)


def build_kernel(nc, bc=BC, nt=NT):
    """Emit the per-core IR. Inputs are declared as DRAM tensors."""
    n_tiles = bc // nt

    stact = nc.dram_tensor("stact", (NUM_LIMBS * 33, bc), F32, kind="ExternalInput").ap()
    w_q1sa = nc.dram_tensor("w_q1sa", (33, 128), F32, kind="ExternalInput").ap()
    w_q1mi = nc.dram_tensor("w_q1mi", (64, 128), F32, kind="ExternalInput").ap()
    b_q1 = nc.dram_tensor("b_q1", (128, 1), F32, kind="ExternalInput").ap()
    w_q2 = nc.dram_tensor("w_q2", (128, 128), F32, kind="ExternalInput").ap()
    b_q2 = nc.dram_tensor("b_q2", (128, 1), F32, kind="ExternalInput").ap()
    w_q3pad = nc.dram_tensor("w_q3pad", (128, 33), F32, kind="ExternalInput").ap()
    w_q3 = nc.dram_tensor("w_q3", (128, 1), F32, kind="ExternalInput").ap()
    b3vec = nc.dram_tensor("b3vec", (33, 1), F32, kind="ExternalInput").ap()
    inj = nc.dram_tensor("inj", (64, 33), F32, kind="ExternalInput").ap()
    w_m1 = nc.dram_tensor("w_m1", (33, 128), F32, kind="ExternalInput").ap()
    b_m1 = nc.dram_tensor("b_m1", (128, 1), F32, kind="ExternalInput").ap()
    w_m2 = nc.dram_tensor("w_m2", (128, 128), F32, kind="ExternalInput").ap()
    b_m2 = nc.dram_tensor("b_m2", (128, 1), F32, kind="ExternalInput").ap()
    w_m3 = nc.dram_tensor("w_m3", (128, 64), F32, kind="ExternalInput").ap()
    b_m3 = nc.dram_tensor("b_m3", (64, 1), F32, kind="ExternalInput").ap()
    ones64 = nc.dram_tensor("ones64", (64, 1), F32, kind="ExternalInput").ap()
    eps24 = nc.dram_tensor("eps24", (1, 1), F32, kind="ExternalInput").ap()
    q_b3_x20 = nc.dram_tensor("q_b3_x20", (1, 1), F32, kind="ExternalInput").ap()
    out = nc.dram_tensor("out", (1, bc), F32, kind="ExternalOutput").ap()

    with tile.TileContext(nc) as tc:
        _emit(tc, nc, n_tiles, nt, stact, w_q1sa, w_q1mi, b_q1, w_q2, b_q2,
              w_q3pad, w_q3, b3vec, inj, w_m1, b_m1, w_m2, b_m2, w_m3, b_m3,
              ones64, eps24, q_b3_x20, out)
    nc.compile()
    return nc


def _emit(tc, nc, n_tiles, nt, stact, w_q1sa, w_q1mi, b_q1, w_q2, b_q2,
          w_q3pad, w_q3, b3vec, inj, w_m1, b_m1, w_m2, b_m2, w_m3, b_m3,
          ones64, eps24, q_b3_x20, out):
    from contextlib import ExitStack
    ctx = ExitStack()
    with ctx:
        wpool = ctx.enter_context(tc.tile_pool(name="weights", bufs=1))
        spool = ctx.enter_context(tc.tile_pool(name="stact", bufs=44))
        hpool = ctx.enter_context(tc.tile_pool(name="acts", bufs=3))
        mdpool = ctx.enter_context(tc.tile_pool(name="md", bufs=14))
        smpool = ctx.enter_context(tc.tile_pool(name="smalls", bufs=2))
        ps_h = ctx.enter_context(tc.tile_pool(name="ps_h", bufs=3, space="PSUM"))
        ps_xm = ctx.enter_context(tc.tile_pool(name="ps_xm", bufs=2, space="PSUM"))
        ps_md = ctx.enter_context(tc.tile_pool(name="ps_md", bufs=2, space="PSUM"))
        ps_xs = ctx.enter_context(tc.tile_pool(name="ps_xs", bufs=1, space="PSUM"))

        # ---- load weights/constants (once) ----
        def wload(ap, shape):
            t = wpool.tile(list(shape), F32, tag=f"w{ap.name}")
            nc.sync.dma_start(t[:], ap[:])
            return t

        W = {}
        W["q1sa"] = wload(w_q1sa, (33, 128))
        W["q1mi"] = wload(w_q1mi, (64, 128))
        W["bq1"] = wload(b_q1, (128, 1))
        W["q2"] = wload(w_q2, (128, 128))
        W["bq2"] = wload(b_q2, (128, 1))
        W["q3pad"] = wload(w_q3pad, (128, 33))
        W["q3"] = wload(w_q3, (128, 1))
        W["b3vec"] = wload(b3vec, (33, 1))
        W["inj"] = wload(inj, (64, 33))
        W["m1"] = wload(w_m1, (33, 128))
        W["bm1"] = wload(b_m1, (128, 1))
        W["m2"] = wload(w_m2, (128, 128))
        W["bm2"] = wload(b_m2, (128, 1))
        W["m3"] = wload(w_m3, (128, 64))
        W["bm3"] = wload(b_m3, (64, 1))
        W["ones64"] = wload(ones64, (64, 1))
        W["eps24"] = wload(eps24, (1, 1))
        W["qb3x20"] = wload(q_b3_x20, (1, 1))

        bc = n_tiles * nt

        AF = mybir.ActivationFunctionType
        ALU = mybir.AluOpType

        for t in range(n_tiles):
            cs = slice(t * nt, (t + 1) * nt)
            # stage this batch-tile's per-limb [state.T; action] slabs
            st_sb = []
            for i in range(NUM_LIMBS):
                s = spool.tile([33, nt], F32, tag="st")
                nc.sync.dma_start(s[:], stact[33 * i:33 * i + 33, cs])
                st_sb.append(s)
            md_sb = {}
            ps_xsum = ps_xs.tile([1, nt], F32, tag="xsum")
            n_emitted = 0

            for d, limbs in enumerate(DEPTHS):
                # ---------- Q chains for this depth ----------
                h2s = {}
                for i in limbs:
                    p = PARENTS[i]
                    c = CHILD_IDX[i]
                    # q1: h1_raw = W1_sa^T @ stact_i (+ W1_mi^T @ mi)
                    p_h1 = ps_h.tile([128, nt], F32, tag="psh")
                    nc.tensor.matmul(p_h1[:], W["q1sa"][:], st_sb[i][:],
                                     start=True, stop=(p < 0))
                    if p >= 0:
                        nc.tensor.matmul(
                            p_h1[:], W["q1mi"][32 * c:32 * c + 32, :],
                            md_sb[p][32 * c:32 * c + 32, :],
                            start=False, stop=True)
                    h1 = hpool.tile([128, nt], F32, tag="h1")
                    nc.scalar.activation(h1[:], p_h1[:], AF.Relu, bias=W["bq1"][:, 0:1])
                    # q2
                    p_h2 = ps_h.tile([128, nt], F32, tag="psh")
                    nc.tensor.matmul(p_h2[:], W["q2"][:], h1[:], start=True, stop=True)
                    h2 = hpool.tile([128, nt], F32, tag="h2")
                    nc.scalar.activation(h2[:], p_h2[:], AF.Relu, bias=W["bq2"][:, 0:1])
                    h2s[i] = h2
                    # q3b: accumulate sum_i q3^T h2_i into xsum psum
                    nc.tensor.matmul(ps_xsum[:], W["q3"][:], h2[:],
                                     start=(n_emitted == 0),
                                     stop=(n_emitted == NUM_LIMBS - 1))
                    n_emitted += 1

                # ---------- xm + M chains (non-leaf limbs only) ----------
                for i in limbs:
                    if i not in NON_LEAF:
                        continue
                    p = PARENTS[i]
                    c = CHILD_IDX[i]
                    # xm_raw psum [33, nt]: row0 = x, rows1-32 = mi
                    p_xm = ps_xm.tile([33, nt], F32, tag="pxm")
                    if p >= 0:
                        nc.tensor.matmul(
                            p_xm[:], W["inj"][32 * c:32 * c + 32, :],
                            md_sb[p][32 * c:32 * c + 32, :],
                            start=True, stop=False)
                    # x into row 0 (w_q3pad col0=q_w3, cols1-32 = 0)
                    nc.tensor.matmul(p_xm[:], W["q3pad"][:], h2s[i][:],
                                     start=(p < 0), stop=True)
                    xm = hpool.tile([33, nt], F32, tag="xm")
                    nc.scalar.activation(xm[:], p_xm[:], AF.Tanh, bias=W["b3vec"][:, 0:1])
                    # m1
                    p_g1 = ps_h.tile([128, nt], F32, tag="psh")
                    nc.tensor.matmul(p_g1[:], W["m1"][:], xm[:], start=True, stop=True)
                    g1 = hpool.tile([128, nt], F32, tag="g1")
                    nc.vector.tensor_scalar(g1[:], p_g1[:], W["bm1"][:, 0:1], 0.0,
                                            op0=ALU.add, op1=ALU.max)
                    # m2
                    p_g2 = ps_h.tile([128, nt], F32, tag="psh")
                    nc.tensor.matmul(p_g2[:], W["m2"][:], g1[:], start=True, stop=True)
                    g2 = hpool.tile([128, nt], F32, tag="g2")
                    nc.vector.tensor_scalar(g2[:], p_g2[:], W["bm2"][:, 0:1], 0.0,
                                            op0=ALU.add, op1=ALU.max)
                    # m3 -> md_raw [64, nt]
                    p_md = ps_md.tile([64, nt], F32, tag="pmd")
                    nc.tensor.matmul(p_md[:], W["m3"][:], g2[:], start=True, stop=True)
                    # + m_b3 bias -> sbuf (unnormalized)
                    mdb = hpool.tile([64, nt], F32, tag="mdb")
                    nc.vector.tensor_scalar(mdb[:], p_md[:], W["bm3"][:, 0:1], None,
                                            op0=ALU.add)
                    # sum of squares via ones-matmul
                    sq = hpool.tile([64, nt], F32, tag="sq")
                    nc.scalar.activation(sq[:], mdb[:], AF.Square)
                    p_ss = ps_xm.tile([33, nt], F32, tag="pxm")
                    nc.tensor.matmul(p_ss[0:1, :], W["ones64"][:], sq[:],
                                     start=True, stop=True)
                    # inv = 1 / sqrt(ss + 1e-24)
                    nrm = smpool.tile([1, nt], F32, tag="nrm")
                    nc.scalar.activation(nrm[:], p_ss[0:1, :], AF.Sqrt,
                                         bias=W["eps24"][:, 0:1])
                    inv = smpool.tile([1, nt], F32, tag="inv")
                    nc.vector.reciprocal_approx_fast(out=inv[:], in_=nrm[:])
                    invb = hpool.tile([64, nt], F32, tag="invb")
                    nc.gpsimd.partition_broadcast(invb[:], inv[:])
                    md = mdpool.tile([64, nt], F32, tag="md")
                    nc.vector.tensor_tensor(md[:], mdb[:], invb[:], op=ALU.mult)
                    md_sb[i] = md

            # finalize: out = xsum + 20*q_b3
            orow = smpool.tile([1, nt], F32, tag="orow")
            nc.scalar.activation(orow[:], ps_xsum[:], AF.Identity,
                                 bias=W["qb3x20"][:, 0:1])
            nc.sync.dma_start(out[0:1, cs], orow[:])


# ---------------------------------------------------------------- host side

def _prep_inputs(state, action, q_w1, q_b1, q_w2, q_b2, q_w3, q_b3,
                 m_w1, m_b1, m_w2, m_b2, m_w3, m_b3, bc):
    """Build per-core input maps (numpy only)."""
    B = state.shape[0]
    n_cores = B // bc
    f32 = np.float32

    # [B, 20, 33]: per limb 32 state dims + action
    stact = np.concatenate(
        [np.asarray(state, f32).reshape(B, NUM_LIMBS, STATE_DIM),
         np.ascontiguousarray(np.asarray(action, f32).T).reshape(B, NUM_LIMBS, 1)],
        axis=2)
    stact_T = np.ascontiguousarray(stact.reshape(B, NUM_LIMBS * 33).T)  # [660, B]

    q_w1 = np.asarray(q_w1, f32)
    w_q1sa = np.ascontiguousarray(q_w1[0:33])                    # [33,128]
    w_q1mi = np.ascontiguousarray(np.tile(q_w1[33:65], (2, 1)))  # [64,128]
    w_q3pad = np.zeros((128, 33), f32)
    w_q3pad[:, 0] = np.asarray(q_w3, f32)[:, 0]
    b3vec = np.zeros((33, 1), f32)
    b3vec[0, 0] = np.float32(q_b3[0])
    inj = np.zeros((64, 33), f32)
    for r in range(2):
        for k in range(32):
            inj[32 * r + k, k + 1] = 1.0
    shared = {
        "w_q1sa": w_q1sa,
        "w_q1mi": w_q1mi,
        "b_q1": np.asarray(q_b1, f32).reshape(128, 1),
        "w_q2": np.ascontiguousarray(np.asarray(q_w2, f32)),
        "b_q2": np.asarray(q_b2, f32).reshape(128, 1),
        "w_q3pad": w_q3pad,
        "w_q3": np.ascontiguousarray(np.asarray(q_w3, f32)),
        "b3vec": b3vec,
        "inj": inj,
        "w_m1": np.ascontiguousarray(np.asarray(m_w1, f32)),
        "b_m1": np.asarray(m_b1, f32).reshape(128, 1),
        "w_m2": np.ascontiguousarray(np.asarray(m_w2, f32)),
        "b_m2": np.asarray(m_b2, f32).reshape(128, 1),
        "w_m3": np.ascontiguousarray(np.asarray(m_w3, f32)),
        "b_m3": np.asarray(m_b3, f32).reshape(64, 1),
        "ones64": np.ones((64, 1), f32),
        "eps24": np.full((1, 1), 1e-24, f32),
        "q_b3_x20": np.full((1, 1), NUM_LIMBS * np.float64(q_b3[0]), f32),
    }
    in_maps = []
    for k in range(n_cores):
        m = dict(shared)
        m["stact"] = np.ascontiguousarray(stact_T[:, k * bc:(k + 1) * bc])
        in_maps.append(m)
    return in_maps


_NC_CACHE = {}


def _get_nc(bc, nt):
    key = (bc, nt)
    if key not in _NC_CACHE:
        nc = bacc.Bacc("TRN2", target_bir_lowering=False, debug=False,
                       enable_asserts=False, num_devices=BATCH // bc)
        _NC_CACHE[key] = build_kernel(nc, bc, nt)
    return _NC_CACHE[key]


def run(inputs, trace=False, bc=BC, nt=NT):
    """Run on hardware across 8 cores; returns (output [B,1], BassKernelResults)."""
    n_cores = BATCH // bc
    nc = _get_nc(bc, nt)
    in_maps = _prep_inputs(bc=bc, **inputs)
    res = bass_utils.run_bass_kernel_spmd(nc, in_maps,
                                          core_ids=list(range(n_cores)),
                                          trace=trace)
    out = np.concatenate([res.results[k]["out"][0] for k in range(n_cores)])
    return out.reshape(-1, 1).astype(np.float32), res


def kernel(**inputs) -> np.ndarray:
    out, _ = run(inputs, trace=False)
    return out
